# revision 27
# baseline (speedup 1.0000x reference)
"""MoE noisy-top2 routing (ChannelClustering) Trainium2 kernel.

Data-parallel over tokens: N=8192 tokens sharded 1024/core across 8 cores.
Per core:
  - router (f32 on PE: selection must match f32 reference bit-near-exactly)
  - noisy top-2 gating computed with mask/max tricks on DVE/ACT
  - dense per-expert Linear+ReLU in bf16 on PE, combined with f32 gates
  - partial channel-similarity mask (g_b @ g_b^T summed over local b-blocks)
Host: shard/layout prep (transposes, casts), gather + final mask mean.
"""

import numpy as np
import ml_dtypes
from contextlib import ExitStack

import concourse.bass as bass
from concourse import bacc
import concourse.mybir as mybir
import concourse.tile as tile
from concourse.bass import ts, ds
from concourse.bass_utils import run_bass_kernel_spmd
from concourse.masks import make_identity

B, C, L, D, E = 64, 128, 1024, 1024, 8
H = D // 4          # 256
H2 = 2 * H          # 512 (gate+noise routers fused)
N = B * C           # 8192 tokens
NCORES = 8
TOK = N // NCORES   # 1024 tokens per core
MT = TOK // 128     # 8 token tiles per core
KT = L // 128       # 8 contraction tiles

F32 = mybir.dt.float32
BF16 = mybir.dt.bfloat16
AF = mybir.ActivationFunctionType
ALU = mybir.AluOpType
AX = mybir.AxisListType

_BUILD_CACHE = {}


def _build(b2_nz: bool, be_nz: bool, wh_ident: bool, debug: bool = False):
    nc = bacc.Bacc(trn_type="TRN2", name="moe_cc")

    xTbf = nc.dram_tensor("xTbf", [L, TOK], BF16, kind="ExternalInput")
    xTlo = nc.dram_tensor("xTlo", [L, TOK], BF16, kind="ExternalInput")
    w1h = nc.dram_tensor("w1h", [L, H2], BF16, kind="ExternalInput")
    w1l = nc.dram_tensor("w1l", [L, H2], BF16, kind="ExternalInput")
    b1d = nc.dram_tensor("b1d", [128, H2 // 128], F32, kind="ExternalInput")
    w2t = nc.dram_tensor("w2t", [H2, 2 * E], F32, kind="ExternalInput")
    b2d = nc.dram_tensor("b2d", [128, 2 * E], F32, kind="ExternalInput")
    nzd = nc.dram_tensor("nzd", [TOK, E], F32, kind="ExternalInput")
    whd = nc.dram_tensor("whd", [E, E], F32, kind="ExternalInput")
    wet = nc.dram_tensor("wet", [E, L, D], BF16, kind="ExternalInput")
    if be_nz:
        beb = nc.dram_tensor("beb", [128, E, D], F32, kind="ExternalInput")
    y_o = nc.dram_tensor("y", [TOK, D], F32, kind="ExternalOutput")
    mk_o = nc.dram_tensor("mask", [C, C], F32, kind="ExternalOutput")
    if debug:
        lg_o = nc.dram_tensor("lg_dbg", [TOK, E], F32, kind="ExternalOutput")
        gt_o = nc.dram_tensor("gt_dbg", [TOK, E], F32, kind="ExternalOutput")

    HM = H2 // 128  # 4 output-partition tiles for router L1

    with tile.TileContext(nc) as tc, ExitStack() as ctx:
        const = ctx.enter_context(tc.tile_pool(name="const", bufs=1))
        xpool = ctx.enter_context(tc.tile_pool(name="xpool", bufs=1))
        hpool = ctx.enter_context(tc.tile_pool(name="hpool", bufs=1))
        gpool = ctx.enter_context(tc.tile_pool(name="gpool", bufs=1))
        ypool = ctx.enter_context(tc.tile_pool(name="ypool", bufs=1))

        # ---- inputs to SBUF ----
        # per-kt loads, kt-interleaved and spread over 4 engine DGE queues so
        # the first R1 matmul only waits for the first kt slices
        w1h_sb = const.tile([128, KT, H2], BF16)
        w1l_sb = const.tile([128, KT, H2], BF16)
        xbf = xpool.tile([128, KT, TOK], BF16)
        xlo = xpool.tile([128, KT, TOK], BF16)
        for kt in range(KT):
            nc.sync.dma_start(w1h_sb[:, kt], w1h[ts(kt, 128), :])
            nc.scalar.dma_start(xbf[:, kt], xTbf[ts(kt, 128), :])
            nc.gpsimd.dma_start(w1l_sb[:, kt], w1l[ts(kt, 128), :])
            nc.gpsimd.dma_start(xlo[:, kt], xTlo[ts(kt, 128), :])
        b1_sb = const.tile([128, HM], F32)
        nc.scalar.dma_start(b1_sb, b1d[:, :])
        w2_sb = const.tile([128, H2 // 128, 2 * E], F32)
        nc.sync.dma_start(w2_sb, w2t.rearrange("(kt p) e -> p kt e", p=128))
        nz_sb = const.tile([128, MT, E], F32)
        nc.sync.dma_start(nz_sb, nzd.rearrange("(mt p) e -> p mt e", p=128))
        ident = const.tile([128, 128], F32)
        make_identity(nc, ident)
        if b2_nz:
            b2_sb = const.tile([128, 2 * E], F32)
            nc.sync.dma_start(b2_sb, b2d[:, :])
        if not wh_ident:
            wh_sb = const.tile([E, E], F32)
            nc.sync.dma_start(wh_sb, whd[:, :])
        if be_nz:
            beb_sb = const.tile([128, E, D], F32)
            nc.sync.dma_start(beb_sb, beb[:, :, :])

        h1T = hpool.tile([128, HM, TOK], F32)   # relu(x @ W1^T)^T : [H2, TOK]
        gates = gpool.tile([128, MT, E], F32)   # [tok, E] per m-tile
        lgts = gpool.tile([128, MT, E], F32)
        y_acc = ypool.tile([128, MT, D], F32)

        # =============== ROUTER ===============
        with tc.tile_pool(name="ps_r1", bufs=1, space="PSUM") as ps_r1:
            # L1: h1T[h, t] = sum_l W1[h, l] x[t, l], computed to ~1e-5
            # accuracy as 3 bf16 products: Wh*xh + Wh*xl + Wl*xh.
            # kt-outer with all 8 PSUM groups open so the first matmul only
            # needs the first kt slice of the input DMAs.
            groups = [(hm, nt) for hm in range(HM) for nt in range(TOK // 512)]
            pss = {g: ps_r1.tile([128, 512], F32, name=f"psr1_{g[0]}_{g[1]}")
                   for g in groups}
            for kt in range(KT):
                for ti in range(3):
                    for hm, nt in groups:
                        if ti == 0:
                            lhsT = w1h_sb[:, kt, ts(hm, 128)]
                            rhs = xbf[:, kt, ts(nt, 512)]
                        elif ti == 1:
                            lhsT = w1h_sb[:, kt, ts(hm, 128)]
                            rhs = xlo[:, kt, ts(nt, 512)]
                        else:
                            lhsT = w1l_sb[:, kt, ts(hm, 128)]
                            rhs = xbf[:, kt, ts(nt, 512)]
                        nc.tensor.matmul(
                            pss[(hm, nt)],
                            lhsT,
                            rhs,
                            start=(kt == 0 and ti == 0),
                            stop=(kt == KT - 1 and ti == 2),
                        )
            for hm, nt in groups:
                # relu(h + b1); b1 varies along partitions (H) here
                nc.scalar.activation(
                    h1T[:, hm, ts(nt, 512)], pss[(hm, nt)], AF.Relu,
                    bias=b1_sb[:, hm : hm + 1],
                )

        with (
            tc.tile_pool(name="ps_r2", bufs=4, space="PSUM") as ps_r2,
            tc.tile_pool(name="ps_tr", bufs=2, space="PSUM") as ps_tr,
            tc.tile_pool(name="rsm", bufs=1) as rsm,
        ):
            # L2: pre[t, 0:8]=clean, pre[t, 8:16]=raw_std (block-diag W2)
            pre3 = rsm.tile([128, MT, 2 * E], F32)
            for m in range(MT):
                ps2 = ps_r2.tile([128, 2 * E], F32, tag="ps2")
                for kt in range(H2 // 128):
                    nc.tensor.matmul(
                        ps2,
                        h1T[:, kt, ts(m, 128)],
                        w2_sb[:, kt, :],
                        start=(kt == 0),
                        stop=(kt == H2 // 128 - 1),
                    )
                if b2_nz:
                    nc.vector.tensor_add(pre3[:, m, :], ps2, b2_sb)
                else:
                    nc.vector.tensor_copy(pre3[:, m, :], ps2)

            # ---- all remaining router math as batched [128, MT, E] ops ----
            def bc(t):  # broadcast a [128, MT] per-(token) scalar over E
                return t[:, :, None].to_broadcast((128, MT, E))

            clean = pre3[:, :, 0:E]
            raw = pre3[:, :, E : 2 * E]
            # noise_std = softplus(raw) + 0.01 ; softplus = ln(1 + exp(x))
            expr = rsm.tile([128, MT, E], F32)
            nc.scalar.activation(expr, raw, AF.Exp)
            sp = rsm.tile([128, MT, E], F32)
            nc.scalar.activation(sp, expr, AF.Ln, bias=1.0)
            # t2 = clean + noise * (sp + 0.01)
            t2 = rsm.tile([128, MT, E], F32)
            nc.vector.scalar_tensor_tensor(
                t2, sp, 0.01, nz_sb, op0=ALU.add, op1=ALU.mult
            )
            if wh_ident:
                nc.vector.tensor_add(lgts, t2, clean)
            else:
                t2c = rsm.tile([128, MT, E], F32)
                nc.vector.tensor_add(t2c, t2, clean)
                for m in range(MT):
                    pst = ps_tr.tile([E, 128], F32, tag="pst")
                    nc.tensor.transpose(pst, t2c[:, m, :], ident)
                    t2T = rsm.tile([E, 128], F32, tag="t2T", bufs=2)
                    nc.vector.tensor_copy(t2T, pst)
                    psl = ps_r2.tile([128, E], F32, tag="psl", bufs=2)
                    nc.tensor.matmul(psl, t2T, wh_sb, start=True, stop=True)
                    nc.vector.tensor_copy(lgts[:, m, :], psl)

            # ---- softmax + top-2 gates (selection from logits) ----
            mx1 = rsm.tile([128, MT], F32)
            nc.vector.reduce_max(mx1, lgts, axis=AX.X)
            lgs = rsm.tile([128, MT, E], F32)
            nc.vector.tensor_sub(lgs, lgts, bc(mx1))
            eprob = rsm.tile([128, MT, E], F32)
            nc.scalar.activation(eprob, lgs, AF.Exp)
            ssum = rsm.tile([128, MT], F32)
            nc.vector.reduce_sum(ssum, eprob, axis=AX.X)
            rs = rsm.tile([128, MT], F32)
            nc.vector.reciprocal(rs, ssum)
            probs = rsm.tile([128, MT, E], F32)
            nc.vector.tensor_mul(probs, eprob, bc(rs))

            is1 = rsm.tile([128, MT, E], F32)
            nc.vector.tensor_tensor(is1, lgts, bc(mx1), ALU.is_equal)
            lmsk = rsm.tile([128, MT, E], F32)
            nc.vector.scalar_tensor_tensor(
                lmsk, is1, -1e30, lgts, op0=ALU.mult, op1=ALU.add
            )
            mx2 = rsm.tile([128, MT], F32)
            nc.vector.reduce_max(mx2, lmsk, axis=AX.X)
            is2 = rsm.tile([128, MT, E], F32)
            nc.vector.tensor_tensor(is2, lmsk, bc(mx2), ALU.is_equal)

            v1 = rsm.tile([128, MT], F32)
            nc.vector.reduce_max(v1, probs, axis=AX.X)
            pmsk = rsm.tile([128, MT, E], F32)
            nc.vector.scalar_tensor_tensor(
                pmsk, is1, -1e30, probs, op0=ALU.mult, op1=ALU.add
            )
            v2 = rsm.tile([128, MT], F32)
            nc.vector.reduce_max(v2, pmsk, axis=AX.X)
            den = rsm.tile([128, MT], F32)
            nc.vector.scalar_tensor_tensor(
                den, v1, 1e-6, v2, op0=ALU.add, op1=ALU.add
            )
            rd = rsm.tile([128, MT], F32)
            nc.vector.reciprocal(rd, den)

            g1 = rsm.tile([128, MT, E], F32)
            nc.vector.tensor_mul(g1, is1, bc(v1))
            g2t = rsm.tile([128, MT, E], F32)
            nc.vector.tensor_mul(g2t, is2, bc(v2))
            nc.vector.tensor_add(gates, g1, g2t)
            nc.vector.tensor_mul(gates, gates, bc(rd))

        # =============== EXPERTS (dense, bf16) ===============
        # mask phase is emitted at the tail, after the last expert matmuls,
        # where it overlaps the ACT/DVE/DMA drain
        with (
            tc.tile_pool(name="wpool", bufs=3) as wpool,
            tc.tile_pool(name="ps_e", bufs=5, space="PSUM") as ps_e,
            tc.tile_pool(name="ps_m", bufs=2, space="PSUM") as ps_m,
            tc.tile_pool(name="ytmp", bufs=4) as ytmp,
            tc.tile_pool(name="gT", bufs=1) as gTp,
        ):
            for e in range(E):
                for nh in range(D // 512):
                    wt = wpool.tile([128, KT, 512], BF16, tag="wt")
                    nc.sync.dma_start(
                        wt,
                        wet[e, :, ds(nh * 512, 512)].rearrange(
                            "(kt p) d -> p kt d", p=128
                        ),
                    )
                    for m in range(MT):
                        ps = ps_e.tile([128, 512], F32, tag="pse")
                        for kt in range(KT):
                            nc.tensor.matmul(
                                ps,
                                xbf[:, kt, ts(m, 128)],
                                wt[:, kt, :],
                                start=(kt == 0),
                                stop=(kt == KT - 1),
                            )
                        if be_nz:
                            nc.vector.tensor_add(
                                ps, ps, beb_sb[:, e, ds(nh * 512, 512)]
                            )
                        # ACT does plain relu (never waits on gates); DVE
                        # applies the per-token gate and accumulates
                        gcol = gates[:, m, e : e + 1]
                        ydst = y_acc[:, m, ds(nh * 512, 512)]
                        yt = ytmp.tile([128, 512], F32, tag="yt")
                        nc.scalar.activation(yt, ps, AF.Relu)
                        if e == 0:
                            nc.vector.tensor_scalar_mul(ydst, yt, gcol)
                        else:
                            nc.vector.scalar_tensor_tensor(
                                ydst, yt, gcol, ydst, op0=ALU.mult, op1=ALU.add
                            )
                        if e == E - 1:
                            # stream each finished y tile out immediately
                            nc.sync.dma_start(
                                y_o[ts(m, 128), ds(nh * 512, 512)], ydst
                            )

                if e == E - 1:
                    # ===== CHANNEL MASK (each m-tile == one batch row b):
                    # partial mask = sum_b g_b @ g_b^T.  gT zero-padded to
                    # K=128: matmuls with K<128 crash on HW.
                    gT = gTp.tile([128, MT, 128], F32)
                    nc.vector.memset(gT, 0.0)
                    for m in range(MT):
                        pst2 = ps_m.tile([E, 128], F32, tag="pst2")
                        nc.tensor.transpose(pst2, gates[:, m, :], ident)
                        nc.vector.tensor_copy(gT[:E, m, :], pst2)
                    psmk = ps_m.tile([128, C], F32, tag="psmk", bufs=1)
                    for m in range(MT):
                        nc.tensor.matmul(
                            psmk,
                            gT[:, m, :],
                            gT[:, m, :],
                            start=(m == 0),
                            stop=(m == MT - 1),
                        )
                    mk_sb = gTp.tile([128, C], F32)
                    nc.vector.tensor_copy(mk_sb, psmk)
                    nc.sync.dma_start(mk_o[:, :], mk_sb)

        if debug:
            nc.sync.dma_start(lg_o.rearrange("(mt p) e -> p mt e", p=128), lgts)
            nc.sync.dma_start(gt_o.rearrange("(mt p) e -> p mt e", p=128), gates)

    nc.finalize()
    return nc


def _get_nc(cfg):
    if cfg not in _BUILD_CACHE:
        _BUILD_CACHE[cfg] = _build(*cfg)
    return _BUILD_CACHE[cfg]


_LAST_EXEC_NS = None
_LAST_TRACE = None


def kernel(x, noise, Wg1, bg1, Wg2, bg2, Wn1, bn1, Wn2, bn2, W_h, We, be,
           _debug=False, _return_raw=False, _trace=False):
    f32 = np.float32
    x = np.asarray(x, f32)
    noise = np.asarray(noise, f32)
    Wg1, Wg2 = np.asarray(Wg1, f32), np.asarray(Wg2, f32)
    Wn1, Wn2 = np.asarray(Wn1, f32), np.asarray(Wn2, f32)
    bg1, bg2 = np.asarray(bg1, f32), np.asarray(bg2, f32)
    bn1, bn2 = np.asarray(bn1, f32), np.asarray(bn2, f32)
    W_h, We, be = np.asarray(W_h, f32), np.asarray(We, f32), np.asarray(be, f32)

    b2_nz = bool(np.any(bg2 != 0) or np.any(bn2 != 0))
    be_nz = bool(np.any(be != 0))
    wh_ident = bool(np.array_equal(W_h, np.eye(E, dtype=f32)))
    cfg = (b2_nz, be_nz, wh_ident, _debug)
    nc = _get_nc(cfg)

    bf16 = ml_dtypes.bfloat16
    xf = x.reshape(N, L)
    w1t = np.ascontiguousarray(np.concatenate([Wg1, Wn1], 0).T)      # [L, H2]
    w1t_h = w1t.astype(bf16)
    w1t_l = (w1t - w1t_h.astype(f32)).astype(bf16)
    b1vec = np.concatenate([bg1, bn1], 0)                            # [H2]
    b1d = np.ascontiguousarray(b1vec.reshape(H2 // 128, 128).T)      # [128, 4]
    w2t = np.zeros((H2, 2 * E), f32)
    w2t[:H, :E] = Wg2.T
    w2t[H:, E:] = Wn2.T
    b2d = np.tile(np.concatenate([bg2, bn2], 0)[None, :], (128, 1))
    wet = np.ascontiguousarray(We.transpose(0, 2, 1)).astype(bf16)   # [E, L, D]

    in_maps = []
    for i in range(NCORES):
        sl = slice(i * TOK, (i + 1) * TOK)
        xT = np.ascontiguousarray(xf[sl].T)                          # [L, TOK]
        xT_h = xT.astype(bf16)
        xT_l = (xT - xT_h.astype(f32)).astype(bf16)
        m = {
            "xTbf": xT_h,
            "xTlo": xT_l,
            "w1h": w1t_h,
            "w1l": w1t_l,
            "b1d": b1d,
            "w2t": w2t,
            "b2d": b2d,
            "nzd": np.ascontiguousarray(noise[sl]),
            "whd": W_h,
            "wet": wet,
        }
        if be_nz:
            m["beb"] = np.ascontiguousarray(
                np.broadcast_to(be[None, :, :], (128, E, D)), f32
            )
        in_maps.append(m)

    run_kwargs = {}
    if _trace:
        run_kwargs = dict(trace=True, trace_cores=[0])
    res = run_bass_kernel_spmd(
        nc, in_maps, core_ids=list(range(NCORES)), **run_kwargs
    )
    global _LAST_EXEC_NS, _LAST_TRACE
    _LAST_EXEC_NS = res.exec_time_ns
    _LAST_TRACE = res.instructions_and_trace
    if _return_raw:
        return res
    results = res.results
    y = np.concatenate([r["y"] for r in results], 0).astype(f32)
    mask = (
        np.sum([r["mask"].astype(np.float64) for r in results], 0) / B
    ).astype(f32)
    return mask, y


# revision 28
# speedup vs baseline: 1.0205x; 1.0205x over previous
"""MoE noisy-top2 routing (ChannelClustering) Trainium2 kernel.

Data-parallel over tokens: N=8192 tokens sharded 1024/core across 8 cores.
Per core:
  - router (f32 on PE: selection must match f32 reference bit-near-exactly)
  - noisy top-2 gating computed with mask/max tricks on DVE/ACT
  - dense per-expert Linear+ReLU in bf16 on PE, combined with f32 gates
  - partial channel-similarity mask (g_b @ g_b^T summed over local b-blocks)
Host: shard/layout prep (transposes, casts), gather + final mask mean.
"""

import numpy as np
import ml_dtypes
from contextlib import ExitStack

import concourse.bass as bass
from concourse import bacc
import concourse.mybir as mybir
import concourse.tile as tile
from concourse.bass import ts, ds
from concourse.bass_utils import run_bass_kernel_spmd
from concourse.masks import make_identity

B, C, L, D, E = 64, 128, 1024, 1024, 8
H = D // 4          # 256
H2 = 2 * H          # 512 (gate+noise routers fused)
N = B * C           # 8192 tokens
NCORES = 8
TOK = N // NCORES   # 1024 tokens per core
MT = TOK // 128     # 8 token tiles per core
KT = L // 128       # 8 contraction tiles

F32 = mybir.dt.float32
BF16 = mybir.dt.bfloat16
AF = mybir.ActivationFunctionType
ALU = mybir.AluOpType
AX = mybir.AxisListType

_BUILD_CACHE = {}


def _build(b1_nz: bool, b2_nz: bool, be_nz: bool, wh_ident: bool, debug: bool = False):
    nc = bacc.Bacc(trn_type="TRN2", name="moe_cc")

    xTbf = nc.dram_tensor("xTbf", [L, TOK], BF16, kind="ExternalInput")
    xTlo = nc.dram_tensor("xTlo", [L, TOK], BF16, kind="ExternalInput")
    w1h = nc.dram_tensor("w1h", [L, H2], BF16, kind="ExternalInput")
    w1l = nc.dram_tensor("w1l", [L, H2], BF16, kind="ExternalInput")
    b1d = nc.dram_tensor("b1d", [128, H2 // 128], F32, kind="ExternalInput")
    w2t = nc.dram_tensor("w2t", [H2, 2 * E], F32, kind="ExternalInput")
    b2d = nc.dram_tensor("b2d", [128, 2 * E], F32, kind="ExternalInput")
    nzd = nc.dram_tensor("nzd", [TOK, E], F32, kind="ExternalInput")
    whd = nc.dram_tensor("whd", [E, E], F32, kind="ExternalInput")
    wet = nc.dram_tensor("wet", [E, L, D], BF16, kind="ExternalInput")
    if be_nz:
        beb = nc.dram_tensor("beb", [128, E, D], F32, kind="ExternalInput")
    y_o = nc.dram_tensor("y", [TOK, D], F32, kind="ExternalOutput")
    mk_o = nc.dram_tensor("mask", [C, C], F32, kind="ExternalOutput")
    if debug:
        lg_o = nc.dram_tensor("lg_dbg", [TOK, E], F32, kind="ExternalOutput")
        gt_o = nc.dram_tensor("gt_dbg", [TOK, E], F32, kind="ExternalOutput")

    HM = H2 // 128  # 4 output-partition tiles for router L1

    with tile.TileContext(nc) as tc, ExitStack() as ctx:
        const = ctx.enter_context(tc.tile_pool(name="const", bufs=1))
        xpool = ctx.enter_context(tc.tile_pool(name="xpool", bufs=1))
        hpool = ctx.enter_context(tc.tile_pool(name="hpool", bufs=1))
        gpool = ctx.enter_context(tc.tile_pool(name="gpool", bufs=1))
        ypool = ctx.enter_context(tc.tile_pool(name="ypool", bufs=1))

        # ---- inputs to SBUF ----
        # per-kt loads, kt-interleaved and spread over 4 engine DGE queues so
        # the first R1 matmul only waits for the first kt slices
        w1h_sb = const.tile([128, KT, H2], BF16)
        w1l_sb = const.tile([128, KT, H2], BF16)
        xbf = xpool.tile([128, KT, TOK], BF16)
        xlo = xpool.tile([128, KT, TOK], BF16)
        for kt in range(KT):
            nc.sync.dma_start(w1h_sb[:, kt], w1h[ts(kt, 128), :])
            nc.scalar.dma_start(xbf[:, kt], xTbf[ts(kt, 128), :])
            nc.gpsimd.dma_start(w1l_sb[:, kt], w1l[ts(kt, 128), :])
            nc.gpsimd.dma_start(xlo[:, kt], xTlo[ts(kt, 128), :])
        b1_sb = const.tile([128, HM], F32)
        nc.scalar.dma_start(b1_sb, b1d[:, :])
        w2_sb = const.tile([128, H2 // 128, 2 * E], F32)
        nc.sync.dma_start(w2_sb, w2t.rearrange("(kt p) e -> p kt e", p=128))
        nz_sb = const.tile([128, MT, E], F32)
        nc.sync.dma_start(nz_sb, nzd.rearrange("(mt p) e -> p mt e", p=128))
        ident = const.tile([128, 128], F32)
        make_identity(nc, ident)
        if b2_nz:
            b2_sb = const.tile([128, 2 * E], F32)
            nc.sync.dma_start(b2_sb, b2d[:, :])
        if not wh_ident:
            wh_sb = const.tile([E, E], F32)
            nc.sync.dma_start(wh_sb, whd[:, :])
        if be_nz:
            beb_sb = const.tile([128, E, D], F32)
            nc.sync.dma_start(beb_sb, beb[:, :, :])

        h1T = hpool.tile([128, HM, TOK], F32)   # relu(x @ W1^T)^T : [H2, TOK]
        gates = gpool.tile([128, MT, E], F32)   # [tok, E] per m-tile
        lgts = gpool.tile([128, MT, E], F32)
        y_acc = ypool.tile([128, MT, D], F32)

        # =============== ROUTER ===============
        with tc.tile_pool(name="ps_r1", bufs=1, space="PSUM") as ps_r1:
            # L1: h1T[h, t] = sum_l W1[h, l] x[t, l], computed to ~1e-5
            # accuracy as 3 bf16 products: Wh*xh + Wh*xl + Wl*xh.
            # kt-outer with all 8 PSUM groups open so the first matmul only
            # needs the first kt slice of the input DMAs.
            groups = [(hm, nt) for nt in range(TOK // 512) for hm in range(HM)]
            pss = {g: ps_r1.tile([128, 512], F32, name=f"psr1_{g[0]}_{g[1]}")
                   for g in groups}
            for kt in range(KT):
                for ti in range(3):
                    for hm, nt in groups:
                        if ti == 0:
                            lhsT = w1h_sb[:, kt, ts(hm, 128)]
                            rhs = xbf[:, kt, ts(nt, 512)]
                        elif ti == 1:
                            lhsT = w1h_sb[:, kt, ts(hm, 128)]
                            rhs = xlo[:, kt, ts(nt, 512)]
                        else:
                            lhsT = w1l_sb[:, kt, ts(hm, 128)]
                            rhs = xbf[:, kt, ts(nt, 512)]
                        nc.tensor.matmul(
                            pss[(hm, nt)],
                            lhsT,
                            rhs,
                            start=(kt == 0 and ti == 0),
                            stop=(kt == KT - 1 and ti == 2),
                        )
            for i, (hm, nt) in enumerate(groups):
                # relu(h + b1); b1 varies along partitions (H) here.  With
                # b1 == 0 split the relus across ACT and DVE so the PSUM
                # drain after the last K pass is 2x faster.
                if b1_nz:
                    nc.scalar.activation(
                        h1T[:, hm, ts(nt, 512)], pss[(hm, nt)], AF.Relu,
                        bias=b1_sb[:, hm : hm + 1],
                    )
                elif i % 2 == 0:
                    nc.vector.tensor_scalar_max(
                        h1T[:, hm, ts(nt, 512)], pss[(hm, nt)], 0.0
                    )
                else:
                    nc.scalar.activation(
                        h1T[:, hm, ts(nt, 512)], pss[(hm, nt)], AF.Relu
                    )

        with (
            tc.tile_pool(name="ps_r2", bufs=4, space="PSUM") as ps_r2,
            tc.tile_pool(name="ps_tr", bufs=2, space="PSUM") as ps_tr,
            tc.tile_pool(name="rsm", bufs=1) as rsm,
        ):
            # L2: pre[t, 0:8]=clean, pre[t, 8:16]=raw_std (block-diag W2)
            pre3 = rsm.tile([128, MT, 2 * E], F32)
            for m in range(MT):
                ps2 = ps_r2.tile([128, 2 * E], F32, tag="ps2")
                for kt in range(H2 // 128):
                    nc.tensor.matmul(
                        ps2,
                        h1T[:, kt, ts(m, 128)],
                        w2_sb[:, kt, :],
                        start=(kt == 0),
                        stop=(kt == H2 // 128 - 1),
                    )
                if b2_nz:
                    nc.vector.tensor_add(pre3[:, m, :], ps2, b2_sb)
                else:
                    nc.vector.tensor_copy(pre3[:, m, :], ps2)

            # ---- all remaining router math as batched [128, MT, E] ops ----
            def bc(t):  # broadcast a [128, MT] per-(token) scalar over E
                return t[:, :, None].to_broadcast((128, MT, E))

            clean = pre3[:, :, 0:E]
            raw = pre3[:, :, E : 2 * E]
            # noise_std = softplus(raw) + 0.01 ; softplus = ln(1 + exp(x))
            expr = rsm.tile([128, MT, E], F32)
            nc.scalar.activation(expr, raw, AF.Exp)
            sp = rsm.tile([128, MT, E], F32)
            nc.scalar.activation(sp, expr, AF.Ln, bias=1.0)
            # t2 = clean + noise * (sp + 0.01)
            t2 = rsm.tile([128, MT, E], F32)
            nc.vector.scalar_tensor_tensor(
                t2, sp, 0.01, nz_sb, op0=ALU.add, op1=ALU.mult
            )
            if wh_ident:
                nc.vector.tensor_add(lgts, t2, clean)
            else:
                t2c = rsm.tile([128, MT, E], F32)
                nc.vector.tensor_add(t2c, t2, clean)
                for m in range(MT):
                    pst = ps_tr.tile([E, 128], F32, tag="pst")
                    nc.tensor.transpose(pst, t2c[:, m, :], ident)
                    t2T = rsm.tile([E, 128], F32, tag="t2T", bufs=2)
                    nc.vector.tensor_copy(t2T, pst)
                    psl = ps_r2.tile([128, E], F32, tag="psl", bufs=2)
                    nc.tensor.matmul(psl, t2T, wh_sb, start=True, stop=True)
                    nc.vector.tensor_copy(lgts[:, m, :], psl)

            # ---- softmax + top-2 gates (selection from logits) ----
            mx1 = rsm.tile([128, MT], F32)
            nc.vector.reduce_max(mx1, lgts, axis=AX.X)
            lgs = rsm.tile([128, MT, E], F32)
            nc.vector.tensor_sub(lgs, lgts, bc(mx1))
            eprob = rsm.tile([128, MT, E], F32)
            nc.scalar.activation(eprob, lgs, AF.Exp)
            ssum = rsm.tile([128, MT], F32)
            nc.vector.reduce_sum(ssum, eprob, axis=AX.X)
            rs = rsm.tile([128, MT], F32)
            nc.vector.reciprocal(rs, ssum)
            probs = rsm.tile([128, MT, E], F32)
            nc.vector.tensor_mul(probs, eprob, bc(rs))

            is1 = rsm.tile([128, MT, E], F32)
            nc.vector.tensor_tensor(is1, lgts, bc(mx1), ALU.is_equal)
            lmsk = rsm.tile([128, MT, E], F32)
            nc.vector.scalar_tensor_tensor(
                lmsk, is1, -1e30, lgts, op0=ALU.mult, op1=ALU.add
            )
            mx2 = rsm.tile([128, MT], F32)
            nc.vector.reduce_max(mx2, lmsk, axis=AX.X)
            is2 = rsm.tile([128, MT, E], F32)
            nc.vector.tensor_tensor(is2, lmsk, bc(mx2), ALU.is_equal)

            v1 = rsm.tile([128, MT], F32)
            nc.vector.reduce_max(v1, probs, axis=AX.X)
            pmsk = rsm.tile([128, MT, E], F32)
            nc.vector.scalar_tensor_tensor(
                pmsk, is1, -1e30, probs, op0=ALU.mult, op1=ALU.add
            )
            v2 = rsm.tile([128, MT], F32)
            nc.vector.reduce_max(v2, pmsk, axis=AX.X)
            den = rsm.tile([128, MT], F32)
            nc.vector.scalar_tensor_tensor(
                den, v1, 1e-6, v2, op0=ALU.add, op1=ALU.add
            )
            rd = rsm.tile([128, MT], F32)
            nc.vector.reciprocal(rd, den)

            g1 = rsm.tile([128, MT, E], F32)
            nc.vector.tensor_mul(g1, is1, bc(v1))
            g2t = rsm.tile([128, MT, E], F32)
            nc.vector.tensor_mul(g2t, is2, bc(v2))
            nc.vector.tensor_add(gates, g1, g2t)
            nc.vector.tensor_mul(gates, gates, bc(rd))

        # =============== EXPERTS (dense, bf16) ===============
        # mask phase is emitted at the tail, after the last expert matmuls,
        # where it overlaps the ACT/DVE/DMA drain
        with (
            tc.tile_pool(name="wpool", bufs=3) as wpool,
            tc.tile_pool(name="ps_e", bufs=5, space="PSUM") as ps_e,
            tc.tile_pool(name="ps_m", bufs=2, space="PSUM") as ps_m,
            tc.tile_pool(name="ytmp", bufs=4) as ytmp,
            tc.tile_pool(name="gT", bufs=1) as gTp,
        ):
            for e in range(E):
                for nh in range(D // 512):
                    wt = wpool.tile([128, KT, 512], BF16, tag="wt")
                    nc.sync.dma_start(
                        wt,
                        wet[e, :, ds(nh * 512, 512)].rearrange(
                            "(kt p) d -> p kt d", p=128
                        ),
                    )
                    for m in range(MT):
                        ps = ps_e.tile([128, 512], F32, tag="pse")
                        for kt in range(KT):
                            nc.tensor.matmul(
                                ps,
                                xbf[:, kt, ts(m, 128)],
                                wt[:, kt, :],
                                start=(kt == 0),
                                stop=(kt == KT - 1),
                            )
                        if be_nz:
                            nc.vector.tensor_add(
                                ps, ps, beb_sb[:, e, ds(nh * 512, 512)]
                            )
                        # ACT does plain relu (never waits on gates); DVE
                        # applies the per-token gate and accumulates
                        gcol = gates[:, m, e : e + 1]
                        ydst = y_acc[:, m, ds(nh * 512, 512)]
                        yt = ytmp.tile([128, 512], F32, tag="yt")
                        nc.scalar.activation(yt, ps, AF.Relu)
                        if e == 0:
                            nc.vector.tensor_scalar_mul(ydst, yt, gcol)
                        else:
                            nc.vector.scalar_tensor_tensor(
                                ydst, yt, gcol, ydst, op0=ALU.mult, op1=ALU.add
                            )
                        if e == E - 1:
                            # stream each finished y tile out immediately
                            nc.sync.dma_start(
                                y_o[ts(m, 128), ds(nh * 512, 512)], ydst
                            )

                if e == E - 1:
                    # schedule the mask phase after all expert work — the
                    # scheduler otherwise hoists it into the post-router
                    # bubble where it stalls PE on the DVE gate chain
                    tc.cur_priority += 100000
                    # ===== CHANNEL MASK (each m-tile == one batch row b):
                    # partial mask = sum_b g_b @ g_b^T.  gT zero-padded to
                    # K=128: matmuls with K<128 crash on HW.
                    gT = gTp.tile([128, MT, 128], F32)
                    nc.vector.memset(gT, 0.0)
                    for m in range(MT):
                        pst2 = ps_m.tile([E, 128], F32, tag="pst2")
                        nc.tensor.transpose(pst2, gates[:, m, :], ident)
                        nc.vector.tensor_copy(gT[:E, m, :], pst2)
                    psmk = ps_m.tile([128, C], F32, tag="psmk", bufs=1)
                    for m in range(MT):
                        nc.tensor.matmul(
                            psmk,
                            gT[:, m, :],
                            gT[:, m, :],
                            start=(m == 0),
                            stop=(m == MT - 1),
                        )
                    mk_sb = gTp.tile([128, C], F32)
                    nc.vector.tensor_copy(mk_sb, psmk)
                    nc.sync.dma_start(mk_o[:, :], mk_sb)

        if debug:
            nc.sync.dma_start(lg_o.rearrange("(mt p) e -> p mt e", p=128), lgts)
            nc.sync.dma_start(gt_o.rearrange("(mt p) e -> p mt e", p=128), gates)

    nc.finalize()
    return nc


def _get_nc(cfg):
    if cfg not in _BUILD_CACHE:
        _BUILD_CACHE[cfg] = _build(*cfg)
    return _BUILD_CACHE[cfg]


_LAST_EXEC_NS = None
_LAST_TRACE = None


def kernel(x, noise, Wg1, bg1, Wg2, bg2, Wn1, bn1, Wn2, bn2, W_h, We, be,
           _debug=False, _return_raw=False, _trace=False):
    f32 = np.float32
    x = np.asarray(x, f32)
    noise = np.asarray(noise, f32)
    Wg1, Wg2 = np.asarray(Wg1, f32), np.asarray(Wg2, f32)
    Wn1, Wn2 = np.asarray(Wn1, f32), np.asarray(Wn2, f32)
    bg1, bg2 = np.asarray(bg1, f32), np.asarray(bg2, f32)
    bn1, bn2 = np.asarray(bn1, f32), np.asarray(bn2, f32)
    W_h, We, be = np.asarray(W_h, f32), np.asarray(We, f32), np.asarray(be, f32)

    b1_nz = bool(np.any(bg1 != 0) or np.any(bn1 != 0))
    b2_nz = bool(np.any(bg2 != 0) or np.any(bn2 != 0))
    be_nz = bool(np.any(be != 0))
    wh_ident = bool(np.array_equal(W_h, np.eye(E, dtype=f32)))
    cfg = (b1_nz, b2_nz, be_nz, wh_ident, _debug)
    nc = _get_nc(cfg)

    bf16 = ml_dtypes.bfloat16
    xf = x.reshape(N, L)
    w1t = np.ascontiguousarray(np.concatenate([Wg1, Wn1], 0).T)      # [L, H2]
    w1t_h = w1t.astype(bf16)
    w1t_l = (w1t - w1t_h.astype(f32)).astype(bf16)
    b1vec = np.concatenate([bg1, bn1], 0)                            # [H2]
    b1d = np.ascontiguousarray(b1vec.reshape(H2 // 128, 128).T)      # [128, 4]
    w2t = np.zeros((H2, 2 * E), f32)
    w2t[:H, :E] = Wg2.T
    w2t[H:, E:] = Wn2.T
    b2d = np.tile(np.concatenate([bg2, bn2], 0)[None, :], (128, 1))
    wet = np.ascontiguousarray(We.transpose(0, 2, 1)).astype(bf16)   # [E, L, D]

    in_maps = []
    for i in range(NCORES):
        sl = slice(i * TOK, (i + 1) * TOK)
        xT = np.ascontiguousarray(xf[sl].T)                          # [L, TOK]
        xT_h = xT.astype(bf16)
        xT_l = (xT - xT_h.astype(f32)).astype(bf16)
        m = {
            "xTbf": xT_h,
            "xTlo": xT_l,
            "w1h": w1t_h,
            "w1l": w1t_l,
            "b1d": b1d,
            "w2t": w2t,
            "b2d": b2d,
            "nzd": np.ascontiguousarray(noise[sl]),
            "whd": W_h,
            "wet": wet,
        }
        if be_nz:
            m["beb"] = np.ascontiguousarray(
                np.broadcast_to(be[None, :, :], (128, E, D)), f32
            )
        in_maps.append(m)

    run_kwargs = {}
    if _trace:
        run_kwargs = dict(trace=True, trace_cores=[0])
    res = run_bass_kernel_spmd(
        nc, in_maps, core_ids=list(range(NCORES)), **run_kwargs
    )
    global _LAST_EXEC_NS, _LAST_TRACE
    _LAST_EXEC_NS = res.exec_time_ns
    _LAST_TRACE = res.instructions_and_trace
    if _return_raw:
        return res
    results = res.results
    y = np.concatenate([r["y"] for r in results], 0).astype(f32)
    mask = (
        np.sum([r["mask"].astype(np.float64) for r in results], 0) / B
    ).astype(f32)
    return mask, y
